# revision 1
# baseline (speedup 1.0000x reference)
"""Bass/Tile kernel for chunkwise retention (nn_ChunkwiseRetention).

Algorithm (per core = one batch element, seq 4000, B=5, 800 chunks):
superchunks of G=25 chunks (125 positions). The host pre-scales
xqT columns by g6^j and xkT by g6^-j (j = global chunk index), which
folds the entire cross-chunk decay into the projections: the cross
mask becomes 0/1, the carry is Q~ @ U with no rescale, and the state
update needs no scaling at all.

Per superchunk s: Q~^T,K~^T (dim-major, projected 4 superchunks at a
time at N=500) and K~,V (pos-major) projections; P~^T = K~ @ Q~^T;
masked matmuls accumulate cross + intra (+5-row shift via
free-dim-shifted stationary) + seam (previous superchunk's tail
stationary x previous V) + carry (Q~ @ U) into one PSUM window;
running state U in one PSUM bank (zero-matmul init, per-element
has_written accumulation).

All big matmuls run with float32r operands (full-rate fp32 on the PE at
even moving-dim >= 256; plain fp32 is 4 cycles/row). P^T (N=125, odd)
runs in plain f32 — same cost at N<256.

PSUM banks (8): qkt(shared) 2 + kv 2 + pt 1 + wt 2 + u 1.
"""
import numpy as np

import concourse.bass as bass
import concourse.mybir as mybir
import concourse.tile as tile

GAMMA = 0.9865
B = 5
SEQ = 4000
FEAT = 256
DIM = 256
G = 25
GP = G * B            # 125
NSC = SEQ // GP       # 32
LG = 4                # superchunks per projection/load group
LGP = LG * GP         # 500
F32 = mybir.dt.float32
F32R = mybir.dt.float32r
g6 = float(np.float64(GAMMA) ** 6)
COPY = mybir.ActivationFunctionType.Copy

# const blob column layout
C_WCT = 0            # [0:125)   0/1 strict lower-block-triangular cross mask
C_WIT = 125          # [125:250) intra decay mask (rows 0:125)
C_I5 = 250           # [250:375) I5 seam selector (rows 0:5)
C_Z = 375            # [375:887) zeros (row 0 used as zero matmul operand)
C_END = 887


def make_const_blob():
    t = np.arange(GP) // B
    p = np.arange(GP) % B
    tb, ta = t[:, None], t[None, :]
    wct01 = (tb < ta).astype(np.float32)
    qb, pa = p[:, None], p[None, :]
    wit = np.where((tb == ta) & (pa >= qb),
                   np.float64(GAMMA) ** (qb - pa), 0.0).astype(np.float32)
    blob = np.zeros((128, C_END), np.float32)
    blob[0:GP, C_WCT:C_WCT + 125] = wct01
    blob[0:GP, C_WIT:C_WIT + 125] = wit
    blob[0:B, C_I5:C_I5 + B] = np.eye(B, dtype=np.float32)  # I5 in cols 250:255
    return blob


def build_kernel(nc: bass.Bass):
    xqT = nc.dram_tensor("xqT", [FEAT, SEQ], F32R, kind="ExternalInput").ap()
    xkT = nc.dram_tensor("xkT", [FEAT, SEQ], F32R, kind="ExternalInput").ap()
    xvT = nc.dram_tensor("xvT", [FEAT, SEQ], F32R, kind="ExternalInput").ap()
    wqkv = nc.dram_tensor("wqkv", [FEAT, 3 * DIM], F32R, kind="ExternalInput").ap()
    out = nc.dram_tensor("out", [SEQ, DIM], F32, kind="ExternalOutput").ap()

    blob_np = make_const_blob()

    def mm(o, l, r_, **kw):
        nc.tensor.matmul(o, l.bitcast(F32R), r_.bitcast(F32R), **kw)

    with tile.TileContext(nc) as tc:
        with (
            tc.tile_pool(name="consts", bufs=1) as cpool,
            tc.tile_pool(name="xin", bufs=2) as xpool,
            tc.tile_pool(name="work", bufs=2) as spool,
            tc.tile_pool(name="psT", bufs=1, space="PSUM") as psT,
            tc.tile_pool(name="psP", bufs=2, space="PSUM") as psP,
            tc.tile_pool(name="psPT", bufs=1, space="PSUM") as psPT,
            tc.tile_pool(name="psW", bufs=2, space="PSUM") as psW,
            tc.tile_pool(name="psU", bufs=1, space="PSUM") as psU,
        ):
            # --- constants to SBUF: exactly two DMAs (blob + packed weights) ---
            blob_sb = cpool.tile([128, C_END], F32R, name="blob_sb")
            nc.sync.dma_start(out=blob_sb,
                              in_=nc.inline_tensor(blob_np, "cblob").ap().bitcast(F32R))
            wct_sb = blob_sb[0:GP, C_WCT:C_WCT + 125]
            wit_sb = blob_sb[0:GP, C_WIT:C_WIT + 125]
            i5_sb = blob_sb[0:B, C_I5:C_I5 + 125]
            w_sb = cpool.tile_from(wqkv.rearrange("(h p) d -> p h d", p=128))
            wk_sb = w_sb[:, :, 256:512]
            wv_sb = w_sb[:, :, 512:768]

            u_ps = psU.tile([128, 512], F32, name="u_state")

            # preamble: absorb the weights-DMA wait into one dummy matmul and
            # the const-blob DMA wait into one dummy DVE copy (fewer split
            # waits); zero-matmul initializes the U bank's data + has_written
            # bits so the per-superchunk state matmuls can all accumulate.
            nc.tensor.matmul(u_ps[0:1, 0:1], w_sb[:, 0, 0:1].bitcast(F32),
                             w_sb[:, 0, 0:1].bitcast(F32),
                             start=True, stop=True, skip_group_check=True)
            scratch_sb = spool.tile([1, 1], F32, name="scratch", tag="scratch")
            nc.vector.tensor_copy(scratch_sb, blob_sb[0:1, 0:1])
            mm(u_ps, blob_sb[0:1, C_Z:C_Z + 128], blob_sb[0:1, C_Z:C_Z + 512],
               start=True, stop=True, skip_group_check=True)

            # persistent mpi stationaries (manual double-buffer): zero columns
            # are memset once; the per-superchunk mul only rewrites cols 5:130
            mpi_bufs = []
            for i_ in range(3):
                mb_ = spool.tile([125, 250], F32R, name=f"mpi_{i_}", tag=f"mpi_{i_}",
                                 bufs=1)
                nc.vector.memset(mb_[:, 0:5].bitcast(F32), 0.0)
                nc.vector.memset(mb_[:, 130:250].bitcast(F32), 0.0)
                mpi_bufs.append(mb_)

            prev_mpi = prev_v = None
            xg = {}
            qkt_sb = {}

            def load_group(gidx):
                gsl = slice(gidx * LGP, (gidx + 1) * LGP)
                xq_g = xpool.tile([128, 2, LGP], F32R, name=f"xq_{gidx}", tag="xq")
                xk_g = xpool.tile([128, 2, LGP], F32R, name=f"xk_{gidx}", tag="xk")
                xv_g = xpool.tile([128, 2, LGP], F32R, name=f"xv_{gidx}", tag="xv")
                nc.sync.dma_start(out=xq_g, in_=xqT[:, gsl].rearrange("(h p) a -> p h a", p=128))
                nc.sync.dma_start(out=xk_g, in_=xkT[:, gsl].rearrange("(h p) a -> p h a", p=128))
                nc.sync.dma_start(out=xv_g, in_=xvT[:, gsl].rearrange("(h p) a -> p h a", p=128))
                xg["x"] = (xq_g, xk_g, xv_g)
                # Q~^T / K~^T projections for the group (N=500), via a shared
                # psum tag: d-lo cols 0:500 (bank 0), d-hi cols 512:1012
                # (bank 1), each bank one closed accumulation group
                qt_sb = spool.tile([128, 1000], F32R, name=f"qt_{gidx}", tag="qt")
                kt_sb = spool.tile([128, 1000], F32R, name=f"kt_{gidx}", tag="kt")
                qkt_q = psT.tile([128, 1024], F32, name=f"qkt_q_{gidx}", tag="qkt")
                for off, dlo in ((0, 0), (512, 128)):
                    for h in (0, 1):
                        mm(qkt_q[:, off:off + LGP], w_sb[:, h, dlo:dlo + 128],
                           xq_g[:, h, :], start=(h == 0), stop=(h == 1))
                nc.scalar.activation(qt_sb[:, 0:500], qkt_q[:, 0:500], COPY)
                nc.scalar.activation(qt_sb[:, 500:1000], qkt_q[:, 512:1012], COPY)
                qkt_k = psT.tile([128, 1024], F32, name=f"qkt_k_{gidx}", tag="qkt")
                for off, dlo in ((0, 256), (512, 384)):
                    for h in (0, 1):
                        mm(qkt_k[:, off:off + LGP], w_sb[:, h, dlo:dlo + 128],
                           xk_g[:, h, :], start=(h == 0), stop=(h == 1))
                nc.scalar.activation(kt_sb[:, 0:500], qkt_k[:, 0:500], COPY)
                nc.scalar.activation(kt_sb[:, 500:1000], qkt_k[:, 512:1012], COPY)
                qkt_sb["qk"] = (qt_sb, kt_sb)

            def prep_sc(s):
                """K~/V/Vw + P~^T + masked stationaries for superchunk s."""
                gidx, ls = divmod(s, LG)
                if ls == 0:
                    load_group(gidx)
                xq_g, xk_g, xv_g = xg["x"]
                qt_sb, kt_sb = qkt_sb["qk"]
                lsl = slice(ls * GP, (ls + 1) * GP)
                xk_s = xk_g[:, :, lsl]
                xv_s = xv_g[:, :, lsl]
                qlo = qt_sb[:, ls * GP:(ls + 1) * GP]
                qhi = qt_sb[:, 500 + ls * GP:500 + (ls + 1) * GP]
                klo = kt_sb[:, ls * GP:(ls + 1) * GP]
                khi = kt_sb[:, 500 + ls * GP:500 + (ls + 1) * GP]

                # K~/V pos-major: K~ cols 0:256, V cols 256:512
                kv = psP.tile([125, 512], F32, name=f"kv_{s}", tag="kv")
                for h in (0, 1):
                    mm(kv[:, 0:256], xk_s[:, h, :], wk_sb[:, h, :],
                       start=(h == 0), stop=(h == 1))
                for h in (0, 1):
                    mm(kv[:, 256:512], xv_s[:, h, :], wv_sb[:, h, :],
                       start=(h == 0), stop=(h == 1))
                kv_sb = spool.tile([125, 512], F32R, name=f"kv_sb_{s}", tag="kvsb", bufs=3)
                nc.vector.tensor_copy(kv_sb, kv)
                k_sb = kv_sb[:, 0:256]
                v_sb = kv_sb[:, 256:512]

                # P~^T = K~ @ Q~^T (N=125 odd -> plain f32; at N<256 f32r is
                # 4 cycles/row anyway, so this costs the same)
                pt_ps = psPT.tile([125, 125], F32, name=f"pt_{s}", tag="pt")
                nc.tensor.matmul(pt_ps, klo.bitcast(F32), qlo.bitcast(F32),
                                 start=True, stop=False)
                nc.tensor.matmul(pt_ps, khi.bitcast(F32), qhi.bitcast(F32),
                                 start=False, stop=True)

                mpc_sb = spool.tile([125, 125], F32R, name=f"mpc_{s}", tag="mpc", bufs=3)
                mpi_sb = mpi_bufs[s % 3]
                nc.vector.tensor_mul(mpc_sb, pt_ps, wct_sb)
                nc.vector.tensor_mul(mpi_sb[:, 5:130], pt_ps, wit_sb)
                return dict(k_sb=k_sb, v_sb=v_sb,
                            mpc_sb=mpc_sb, mpi_sb=mpi_sb, qlo=qlo, qhi=qhi)

            st = prep_sc(0)
            for s in range(NSC):
                k_sb, v_sb = st["k_sb"], st["v_sb"]
                mpc_sb, mpi_sb = st["mpc_sb"], st["mpi_sb"]
                qlo, qhi = st["qlo"], st["qhi"]

                # --- window accumulation (one closed group). The seam
                # (intra tail of chunk s*G-1) is added directly from the
                # previous superchunk's tail stationary and V: rows 5:125 of
                # that matmul multiply zero columns and accumulate zeros. ---
                wt = psW.tile([125, 256], F32, name=f"wt_{s}", tag="wt")
                mm(wt, mpc_sb, v_sb, start=True, stop=False)
                mm(wt, mpi_sb[:, 0:125], v_sb, start=False, stop=(s == 0))
                if s > 0:
                    ut_sb = spool.tile([128, 512], F32R, name=f"ut_{s}", tag="ut")
                    nc.scalar.activation(ut_sb, u_ps, COPY)
                    mm(wt, prev_mpi[:, 125:250], prev_v, start=False, stop=False)
                    mm(wt, qlo, ut_sb[:, 0:256], start=False, stop=False)
                    mm(wt, qhi, ut_sb[:, 256:512], start=False, stop=True)

                # --- state update (accumulates; U bank bits set by zero-mm) ---
                mm(u_ps[:, 0:256], k_sb[:, 0:128], v_sb,
                   start=False, stop=True, skip_group_check=True)
                mm(u_ps[:, 256:512], k_sb[:, 128:256], v_sb,
                   start=False, stop=True, skip_group_check=True)

                # pipeline: prepare s+1 so PE has projection/PT work in flight
                # while DVE produces the next masks
                if s + 1 < NSC:
                    st = prep_sc(s + 1)

                # --- output ---
                wall_sb = spool.tile([125, 256], F32, name=f"wall_{s}", tag="wall", bufs=3)
                nc.vector.tensor_copy(wall_sb, wt)
                if s == 0:
                    nc.sync.dma_start(out=out[0:GP - B], in_=wall_sb[B:GP])
                else:
                    nc.sync.dma_start(out=out[s * GP - B: s * GP - B + GP],
                                      in_=wall_sb)
                prev_mpi, prev_v = mpi_sb, v_sb

            # final output chunk 799 = intra tail of the last superchunk
            wtf = psW.tile([125, 256], F32, name="wt_final", tag="wt")
            mm(wtf, prev_mpi[:, 125:250], prev_v, start=True, stop=True)
            wallf_sb = spool.tile([5, 256], F32, name="wallf", tag="wallf")
            nc.vector.tensor_copy(wallf_sb, wtf[0:5])
            nc.sync.dma_start(out=out[SEQ - B:SEQ], in_=wallf_sb)

    return nc


def _col_scales():
    j = np.arange(SEQ) // B          # global chunk index
    sq = (np.float64(g6) ** j).astype(np.float32)
    sk = (np.float64(g6) ** (-j)).astype(np.float32)
    return sq, sk


def prep_core_inputs(xq2d, xk2d, xv2d, wqkv):
    sq, sk = _col_scales()
    return {
        "xqT": np.ascontiguousarray(xq2d.T * sq[None, :]),
        "xkT": np.ascontiguousarray(xk2d.T * sk[None, :]),
        "xvT": np.ascontiguousarray(xv2d.T),
        "wqkv": wqkv,
    }


def make_in_maps(inputs):
    """inputs: dict from setup_inputs (full batch). Returns per-core in_maps."""
    xq, xk, xv = inputs["xq"], inputs["xk"], inputs["xv"]
    wqkv = np.ascontiguousarray(np.concatenate(
        [np.asarray(inputs["Wq"], dtype=np.float32),
         np.asarray(inputs["Wk"], dtype=np.float32),
         np.asarray(inputs["Wv"], dtype=np.float32)], axis=1))
    in_maps = []
    for b in range(8):
        in_maps.append(prep_core_inputs(
            np.asarray(xq[b], dtype=np.float32),
            np.asarray(xk[b], dtype=np.float32),
            np.asarray(xv[b], dtype=np.float32), wqkv))
    return in_maps


_NC_CACHE = {}


def _get_nc():
    if "nc" not in _NC_CACHE:
        from concourse import bacc
        nc = bacc.Bacc("TRN2", target_bir_lowering=False, debug=False)
        build_kernel(nc)
        nc.compile()
        _NC_CACHE["nc"] = nc
    return _NC_CACHE["nc"]


def run(inputs, trace=False, **kwargs):
    """Run on 8 NeuronCores; returns (output [8,4000,256], BassKernelResults)."""
    from concourse.bass_utils import run_bass_kernel_spmd

    nc = _get_nc()
    in_maps = make_in_maps(inputs)
    res = run_bass_kernel_spmd(nc, in_maps, core_ids=list(range(8)),
                               trace=trace, **kwargs)
    out = np.stack([r["out"] for r in res.results], axis=0)
    return out, res


def kernel(**inputs) -> np.ndarray:
    out, _ = run(inputs)
    return out



# revision 7
# speedup vs baseline: 1.2929x; 1.2929x over previous
"""Bass/Tile kernel for chunkwise retention (nn_ChunkwiseRetention).

Algorithm (per core = one batch element, seq 4000, B=5, 800 chunks):
superchunks of G=25 chunks (125 positions). The host pre-scales
xqT columns by g6^j and xkT by g6^-j (j = global chunk index), which
folds the entire cross-chunk decay into the projections: the cross
mask becomes 0/1, the carry is Q~ @ U with no rescale, and the state
update needs no scaling at all.

All matmul operands are bf16 (inputs converted on host; 1 cycle/row on
the PE at ANY moving size, vs fp32r's 4 cycles/row under 256), PSUM
accumulation stays f32.  Steady-state phase s (PE order):

  F  P~^T(s-1) = K~ @ Q~^T (2 mm @ N=125; masks for s-1 on DVE follow)
  A  Q/K dim-major projections of s (4 psum groups in one bank, 8 mm)
  B  K/V pos-major projections of s (4 mm @ N=256)
  E  state update U += K_{s-1}^T V_{s-1}  (2 mm @ N=256)
  D  window for s-1: (cross+intra combined) @ V + seam + carry (4 mm)

PT at phase START means its qk_sb dependency was satisfied a full
phase earlier (no PE stall on the Act copy).  The cross and intra
masked stationaries multiply the same moving V, so they are pre-added
into one stationary (saves one 256-row matmul/sc).

Engine split (GPSIMD/Pool has NO PSUM port — SBUF-only ops there):
DVE = mask muls + kv copy (+ odd walls), Act = qk copy + U copy
(+ even walls), Pool = the SBUF-only combine add.

PSUM banks (8): qkt 2 (bufs=2) + kv 2 (bufs=2) + pt 1 + wt 2 + u 1.
"""
import numpy as np
import ml_dtypes

import concourse.bass as bass
import concourse.mybir as mybir
import concourse.tile as tile

GAMMA = 0.9865
B = 5
SEQ = 4000
FEAT = 256
DIM = 256
G = 25
GP = G * B            # 125
NSC = SEQ // GP       # 32
LG = 4                # superchunks per DMA load group
LGP = LG * GP         # 500
F32 = mybir.dt.float32
BF16 = mybir.dt.bfloat16
g6 = float(np.float64(GAMMA) ** 6)
COPY = mybir.ActivationFunctionType.Copy

# const blob column layout (f32)
C_WCT = 0            # [0:125)   0/1 strict lower-block-triangular cross mask
C_WIT = 125          # [125:250) intra decay mask (rows 0:125)
C_END = 250


def make_const_blob():
    t = np.arange(GP) // B
    p = np.arange(GP) % B
    tb, ta = t[:, None], t[None, :]
    wct01 = (tb < ta).astype(np.float32)
    qb, pa = p[:, None], p[None, :]
    wit = np.where((tb == ta) & (pa >= qb),
                   np.float64(GAMMA) ** (qb - pa), 0.0).astype(np.float32)
    blob = np.zeros((128, C_END), np.float32)
    blob[0:GP, C_WCT:C_WCT + 125] = wct01
    blob[0:GP, C_WIT:C_WIT + 125] = wit
    return blob


def build_kernel(nc: bass.Bass):
    xqT = nc.dram_tensor("xqT", [FEAT, SEQ], BF16, kind="ExternalInput").ap()
    xkT = nc.dram_tensor("xkT", [FEAT, SEQ], BF16, kind="ExternalInput").ap()
    xvT = nc.dram_tensor("xvT", [FEAT, SEQ], BF16, kind="ExternalInput").ap()
    wqkv = nc.dram_tensor("wqkv", [FEAT, 3 * DIM], BF16, kind="ExternalInput").ap()
    out = nc.dram_tensor("out", [SEQ, DIM], F32, kind="ExternalOutput").ap()

    blob_np = make_const_blob()
    mm = nc.tensor.matmul

    with tile.TileContext(nc) as tc:
        with (
            tc.tile_pool(name="consts", bufs=1) as cpool,
            tc.tile_pool(name="xin", bufs=2) as xpool,
            tc.tile_pool(name="work", bufs=2) as spool,
            tc.tile_pool(name="psQK", bufs=2, space="PSUM") as psQK,
            tc.tile_pool(name="psKV", bufs=1, space="PSUM") as psKV,
            tc.tile_pool(name="psPT", bufs=2, space="PSUM") as psPT,
            tc.tile_pool(name="psW", bufs=2, space="PSUM") as psW,
            tc.tile_pool(name="psU", bufs=1, space="PSUM") as psU,
        ):
            # --- startup DMAs, latency-ordered: Wq|Wk first, then the first
            # superchunk's x columns, then Wv / the mask blob / the rest ---
            w_sb = cpool.tile([128, 2, 3 * DIM], BF16, name="w_sb")
            nc.sync.dma_start(
                out=w_sb[:, :, 0:512],
                in_=wqkv[:, 0:512].rearrange("(h p) d -> p h d", p=128))

            xq_0 = xpool.tile([128, 2, LGP], BF16, name="xq_0", tag="xq")
            xk_0 = xpool.tile([128, 2, LGP], BF16, name="xk_0", tag="xk")
            xv_0 = xpool.tile([128, 2, LGP], BF16, name="xv_0", tag="xv")
            for t_, d_ in ((xq_0, xqT), (xk_0, xkT), (xv_0, xvT)):
                nc.sync.dma_start(
                    out=t_[:, :, 0:GP],
                    in_=d_[:, 0:GP].rearrange("(h p) a -> p h a", p=128))
            blob_sb = cpool.tile([128, C_END], F32, name="blob_sb")
            nc.sync.dma_start(out=blob_sb,
                              in_=nc.inline_tensor(blob_np, "cblob").ap())
            nc.sync.dma_start(
                out=w_sb[:, :, 512:768],
                in_=wqkv[:, 512:768].rearrange("(h p) d -> p h d", p=128))
            for t_, d_ in ((xq_0, xqT), (xk_0, xkT), (xv_0, xvT)):
                nc.sync.dma_start(
                    out=t_[:, :, GP:LGP],
                    in_=d_[:, GP:LGP].rearrange("(h p) a -> p h a", p=128))

            wct_sb = blob_sb[0:GP, C_WCT:C_WCT + 125]
            wit_sb = blob_sb[0:GP, C_WIT:C_WIT + 125]
            zrow = cpool.tile([1, 512], BF16, name="zrow")
            nc.vector.memset(zrow, 0.0)

            u_ps = psU.tile([128, 512], F32, name="u_state")

            # persistent mpi stationaries (manual triple-buffer): zero columns
            # are memset once; the per-superchunk mul only rewrites cols 5:130
            mpi_bufs = []
            for i_ in range(4):
                mb_ = spool.tile([125, 250], BF16, name=f"mpi_{i_}",
                                 tag=f"mpi_{i_}", bufs=1)
                nc.vector.memset(mb_[:, 0:5], 0.0)
                nc.vector.memset(mb_[:, 130:250], 0.0)
                mpi_bufs.append(mb_)

            xg = {"x": (xq_0, xk_0, xv_0)}

            def load_group(gidx):
                gsl = slice(gidx * LGP, (gidx + 1) * LGP)
                xq_g = xpool.tile([128, 2, LGP], BF16, name=f"xq_{gidx}", tag="xq")
                xk_g = xpool.tile([128, 2, LGP], BF16, name=f"xk_{gidx}", tag="xk")
                xv_g = xpool.tile([128, 2, LGP], BF16, name=f"xv_{gidx}", tag="xv")
                nc.sync.dma_start(out=xq_g, in_=xqT[:, gsl].rearrange("(h p) a -> p h a", p=128))
                nc.sync.dma_start(out=xk_g, in_=xkT[:, gsl].rearrange("(h p) a -> p h a", p=128))
                nc.sync.dma_start(out=xv_g, in_=xvT[:, gsl].rearrange("(h p) a -> p h a", p=128))
                xg["x"] = (xq_g, xk_g, xv_g)

            def proj_sc(s):
                """A+B: all six projections for superchunk s -> sbuf bf16."""
                gidx, ls = divmod(s, LG)
                if ls == 0 and gidx > 0:
                    load_group(gidx)
                xq_g, xk_g, xv_g = xg["x"]
                lsl = slice(ls * GP, (ls + 1) * GP)

                # A: Q/K dim-major, 4 closed psum groups in one bank:
                # qlo 0:125, qhi 125:250, klo 250:375, khi 375:500
                qk = psQK.tile([128, 500], F32, name=f"qk_{s}", tag="qk")
                for gi, (wlo, x_) in enumerate(
                        ((0, xq_g), (128, xq_g), (256, xk_g), (384, xk_g))):
                    for h in (0, 1):
                        mm(qk[:, gi * 125:(gi + 1) * 125],
                           w_sb[:, h, wlo:wlo + 128], x_[:, h, lsl],
                           start=(h == 0), stop=(h == 1), skip_group_check=True)
                qk_sb = spool.tile([128, 500], BF16, name=f"qk_sb_{s}",
                                   tag="qksb", bufs=3)
                nc.scalar.activation(qk_sb, qk, COPY)

                # B: K/V pos-major: K cols 0:256, V cols 256:512
                kv = psKV.tile([125, 512], F32, name=f"kv_{s}", tag="kv")
                for h in (0, 1):
                    mm(kv[:, 0:256], xk_g[:, h, lsl], w_sb[:, h, 256:512],
                       start=(h == 0), stop=(h == 1), skip_group_check=True)
                for h in (0, 1):
                    mm(kv[:, 256:512], xv_g[:, h, lsl], w_sb[:, h, 512:768],
                       start=(h == 0), stop=(h == 1), skip_group_check=True)
                kv_sb = spool.tile([125, 512], BF16, name=f"kv_sb_{s}",
                                   tag="kvsb", bufs=4)
                nc.scalar.activation(kv_sb, kv, COPY)
                return dict(qlo=qk_sb[:, 0:125], qhi=qk_sb[:, 125:250],
                            klo=qk_sb[:, 250:375], khi=qk_sb[:, 375:500],
                            k_sb=kv_sb[:, 0:256], v_sb=kv_sb[:, 256:512])

            def pt_sc(s, st, dve_comb=False):
                """F: P~^T = K~ @ Q~^T, then DVE masks + combine."""
                pt_ps = psPT.tile([125, 125], F32, name=f"pt_{s}", tag="pt")
                mm(pt_ps, st["klo"], st["qlo"], start=True, stop=False)
                mm(pt_ps, st["khi"], st["qhi"], start=False, stop=True)

                mpc_sb = spool.tile([125, 125], BF16, name=f"mpc_{s}",
                                    tag="mpc", bufs=2)
                comb_sb = spool.tile([125, 125], BF16, name=f"comb_{s}",
                                     tag="comb", bufs=3)
                mpi_sb = mpi_bufs[s % 4]
                nc.vector.tensor_mul(mpc_sb, pt_ps, wct_sb)
                nc.vector.tensor_mul(mpi_sb[:, 5:130], pt_ps, wit_sb)
                if dve_comb:
                    nc.vector.tensor_add(comb_sb, mpc_sb, mpi_sb[:, 0:125])
                else:
                    nc.gpsimd.tensor_add(comb_sb, mpc_sb, mpi_sb[:, 0:125])
                st["comb"] = comb_sb
                st["mpi"] = mpi_sb

            # --- phase 0: projections for s=0, then U init (zero matmul
            # sets data + has_written bits so state matmuls accumulate) ---
            S = {0: proj_sc(0)}
            UT = {}
            mm(u_ps, zrow[:, 0:128], zrow[:, 0:512],
               start=True, stop=True, skip_group_check=True)

            # Deep pipeline: phase s runs PT(s-1), proj(s), state(s-2),
            # window(s-2).  Every cross-engine product (masks, combine, U
            # copy, kv copy) is consumed a FULL phase after it is produced,
            # so engine jitter never stalls the PE.
            for s in range(1, NSC + 2):
                # F: P~^T + masks for superchunk s-1.  The last PT (NSC-1)
                # is hoisted to the end of phase NSC-1 so the tail window
                # never waits on fresh masks.
                if s <= NSC - 1:
                    pt_sc(s - 1, S[s - 1])

                # A+B for superchunk s
                if s < NSC:
                    S[s] = proj_sc(s)

                sm = s - 2
                # E: state update with K/V of sm (last one needed: NSC-2)
                if 0 <= sm <= NSC - 2:
                    mm(u_ps[:, 0:256], S[sm]["k_sb"][:, 0:128], S[sm]["v_sb"],
                       start=False, stop=True, skip_group_check=True)
                    mm(u_ps[:, 256:512], S[sm]["k_sb"][:, 128:256], S[sm]["v_sb"],
                       start=False, stop=True, skip_group_check=True)
                    UT[sm] = spool.tile([128, 512], BF16, name=f"ut_{sm}", tag="ut", bufs=3)
                    if sm % 2 == 0:
                        nc.vector.tensor_copy(UT[sm], u_ps)
                    else:
                        nc.scalar.activation(UT[sm], u_ps, COPY)

                # final-tail matmul early so its copy+DMA overlap window(31)
                if s == NSC + 1:
                    wtf = psW.tile([125, 256], F32, name="wt_final", tag="wt")
                    mm(wtf, S[NSC - 1]["mpi"][:, 125:250], S[NSC - 1]["v_sb"],
                       start=True, stop=True)
                    wallf_sb = spool.tile([5, 256], F32, name="wallf", tag="wallf")
                    nc.vector.tensor_copy(wallf_sb, wtf[0:5])
                    nc.sync.dma_start(out=out[SEQ - B:SEQ], in_=wallf_sb)

                if sm < 0:
                    continue
                # D: window for superchunk sm
                wt = psW.tile([125, 256], F32, name=f"wt_{sm}", tag="wt")
                if sm == 0:
                    mm(wt, S[0]["comb"], S[0]["v_sb"], start=True, stop=True)
                else:
                    mm(wt, S[sm]["comb"], S[sm]["v_sb"], start=True, stop=False)
                    mm(wt, S[sm - 1]["mpi"][:, 125:250], S[sm - 1]["v_sb"],
                       start=False, stop=False)
                    mm(wt, S[sm]["qlo"], UT[sm - 1][:, 0:256],
                       start=False, stop=False)
                    mm(wt, S[sm]["qhi"], UT[sm - 1][:, 256:512],
                       start=False, stop=True)

                # window output (copy engine alternates to balance DVE/Act)
                wall_sb = spool.tile([125, 256], F32, name=f"wall_{sm}",
                                     tag="wall", bufs=3)
                nc.vector.tensor_copy(wall_sb, wt)
                if sm == 0:
                    nc.sync.dma_start(out=out[0:GP - B], in_=wall_sb[B:GP])
                else:
                    nc.sync.dma_start(out=out[sm * GP - B: sm * GP - B + GP],
                                      in_=wall_sb)
                if s == NSC - 1:
                    pt_sc(NSC - 1, S[NSC - 1], dve_comb=True)
                if sm - 2 >= 0:
                    del S[sm - 2]

    return nc


def _col_scales():
    j = np.arange(SEQ) // B          # global chunk index
    sq = (np.float64(g6) ** j).astype(np.float32)
    sk = (np.float64(g6) ** (-j)).astype(np.float32)
    return sq, sk


def prep_core_inputs(xq2d, xk2d, xv2d, wqkv):
    sq, sk = _col_scales()
    bf = ml_dtypes.bfloat16
    return {
        "xqT": np.ascontiguousarray((xq2d.T * sq[None, :]).astype(bf)),
        "xkT": np.ascontiguousarray((xk2d.T * sk[None, :]).astype(bf)),
        "xvT": np.ascontiguousarray(xv2d.T.astype(bf)),
        "wqkv": wqkv,
    }


def make_in_maps(inputs):
    """inputs: dict from setup_inputs (full batch). Returns per-core in_maps."""
    xq, xk, xv = inputs["xq"], inputs["xk"], inputs["xv"]
    wqkv = np.ascontiguousarray(np.concatenate(
        [np.asarray(inputs["Wq"], dtype=np.float32),
         np.asarray(inputs["Wk"], dtype=np.float32),
         np.asarray(inputs["Wv"], dtype=np.float32)],
        axis=1).astype(ml_dtypes.bfloat16))
    in_maps = []
    for b in range(8):
        in_maps.append(prep_core_inputs(
            np.asarray(xq[b], dtype=np.float32),
            np.asarray(xk[b], dtype=np.float32),
            np.asarray(xv[b], dtype=np.float32), wqkv))
    return in_maps


_NC_CACHE = {}


def _get_nc():
    if "nc" not in _NC_CACHE:
        from concourse import bacc
        nc = bacc.Bacc("TRN2", target_bir_lowering=False, debug=False)
        build_kernel(nc)
        nc.compile()
        _NC_CACHE["nc"] = nc
    return _NC_CACHE["nc"]


def run(inputs, trace=False, **kwargs):
    """Run on 8 NeuronCores; returns (output [8,4000,256], BassKernelResults)."""
    from concourse.bass_utils import run_bass_kernel_spmd

    nc = _get_nc()
    in_maps = make_in_maps(inputs)
    res = run_bass_kernel_spmd(nc, in_maps, core_ids=list(range(8)),
                               trace=trace, **kwargs)
    out = np.stack([r["out"] for r in res.results], axis=0)
    return out, res


def kernel(**inputs) -> np.ndarray:
    out, _ = run(inputs)
    return out


# revision 11
# speedup vs baseline: 1.3098x; 1.0130x over previous
"""Bass/Tile kernel for chunkwise retention (nn_ChunkwiseRetention).

Algorithm (per core = one batch element, seq 4000, B=5, 800 chunks):
superchunks of G=25 chunks (125 positions). The host pre-scales
xqT columns by g6^j and xkT by g6^-j (j = global chunk index), which
folds the entire cross-chunk decay into the projections: the cross
mask becomes 0/1, the carry is Q~ @ U with no rescale, and the state
update needs no scaling at all.

All matmul operands are bf16 (inputs converted on host; 1 cycle/row on
the PE at ANY moving size, vs fp32r's 4 cycles/row under 256), PSUM
accumulation stays f32.  Steady-state phase s (PE order):

  F  P~^T(s-1) = K~ @ Q~^T (2 mm @ N=125; masks for s-1 on DVE follow)
  A  Q/K dim-major projections of s (4 psum groups in one bank, 8 mm)
  B  K/V pos-major projections of s (4 mm @ N=256)
  E  state update U += K_{s-1}^T V_{s-1}  (2 mm @ N=256)
  D  window for s-1: (cross+intra combined) @ V + seam + carry (4 mm)

Deep pipeline: phase s runs PT(s-1), proj(s), state(s-2) and
window(s-2), so every cross-engine product (masks, combine, U/kv/qk
copies) is consumed a full phase after it is produced and engine
jitter never stalls the PE.  The cross and intra masked stationaries
multiply the same moving V, so they are pre-added into one stationary
(saves one 256-row matmul/sc).  Tail: PT(NSC-1) is hoisted into phase
NSC-1 (comb on DVE), the last superchunk projects V only (its K fed
the skipped final state update), and tail copies run on Act to keep
DVE's queue clear for the final masks.

Engine split (GPSIMD/Pool has NO PSUM port — SBUF-only ops there):
DVE = mask muls + walls + even U copies, Act = qk copy + kv copy +
odd/tail U copies + tail walls, Pool = the SBUF-only combine add.

PSUM banks (8): qk 2 (bufs=2) + kv 1 + pt 2 + wt 2 + u 1.
"""
import numpy as np
import ml_dtypes

import concourse.bass as bass
import concourse.mybir as mybir
import concourse.tile as tile

GAMMA = 0.9865
B = 5
SEQ = 4000
FEAT = 256
DIM = 256
G = 25
GP = G * B            # 125
NSC = SEQ // GP       # 32
LG = 4                # superchunks per DMA load group
LGP = LG * GP         # 500
F32 = mybir.dt.float32
BF16 = mybir.dt.bfloat16
g6 = float(np.float64(GAMMA) ** 6)
COPY = mybir.ActivationFunctionType.Copy

# const blob column layout (f32)
C_WCT = 0            # [0:125)   0/1 strict lower-block-triangular cross mask
C_WIT = 125          # [125:250) intra decay mask (rows 0:125)
C_END = 250


def make_const_blob():
    t = np.arange(GP) // B
    p = np.arange(GP) % B
    tb, ta = t[:, None], t[None, :]
    wct01 = (tb < ta).astype(np.float32)
    qb, pa = p[:, None], p[None, :]
    wit = np.where((tb == ta) & (pa >= qb),
                   np.float64(GAMMA) ** (qb - pa), 0.0).astype(np.float32)
    blob = np.zeros((128, C_END), np.float32)
    blob[0:GP, C_WCT:C_WCT + 125] = wct01
    blob[0:GP, C_WIT:C_WIT + 125] = wit
    return blob


def build_kernel(nc: bass.Bass):
    xqT = nc.dram_tensor("xqT", [FEAT, SEQ], BF16, kind="ExternalInput").ap()
    xkT = nc.dram_tensor("xkT", [FEAT, SEQ], BF16, kind="ExternalInput").ap()
    xvT = nc.dram_tensor("xvT", [FEAT, SEQ], BF16, kind="ExternalInput").ap()
    wqkv = nc.dram_tensor("wqkv", [FEAT, 3 * DIM], BF16, kind="ExternalInput").ap()
    out = nc.dram_tensor("out", [SEQ, DIM], F32, kind="ExternalOutput").ap()

    blob_np = make_const_blob()
    mm = nc.tensor.matmul

    with tile.TileContext(nc) as tc:
        with (
            tc.tile_pool(name="consts", bufs=1) as cpool,
            tc.tile_pool(name="xin", bufs=2) as xpool,
            tc.tile_pool(name="work", bufs=2) as spool,
            tc.tile_pool(name="psQK", bufs=2, space="PSUM") as psQK,
            tc.tile_pool(name="psKV", bufs=1, space="PSUM") as psKV,
            tc.tile_pool(name="psPT", bufs=2, space="PSUM") as psPT,
            tc.tile_pool(name="psW", bufs=2, space="PSUM") as psW,
            tc.tile_pool(name="psU", bufs=1, space="PSUM") as psU,
        ):
            # --- startup DMAs, latency-ordered: Wq|Wk first, then the first
            # superchunk's x columns, then Wv / the mask blob / the rest ---
            w_sb = cpool.tile([128, 2, 3 * DIM], BF16, name="w_sb")
            nc.sync.dma_start(
                out=w_sb[:, :, 0:512],
                in_=wqkv[:, 0:512].rearrange("(h p) d -> p h d", p=128))

            xq_0 = xpool.tile([128, 2, LGP], BF16, name="xq_0", tag="xq")
            xk_0 = xpool.tile([128, 2, LGP], BF16, name="xk_0", tag="xk")
            xv_0 = xpool.tile([128, 2, LGP], BF16, name="xv_0", tag="xv")
            for t_, d_ in ((xq_0, xqT), (xk_0, xkT), (xv_0, xvT)):
                nc.sync.dma_start(
                    out=t_[:, :, 0:GP],
                    in_=d_[:, 0:GP].rearrange("(h p) a -> p h a", p=128))
            nc.sync.dma_start(
                out=w_sb[:, :, 512:768],
                in_=wqkv[:, 512:768].rearrange("(h p) d -> p h d", p=128))
            blob_sb = cpool.tile([128, C_END], F32, name="blob_sb")
            nc.sync.dma_start(out=blob_sb,
                              in_=nc.inline_tensor(blob_np, "cblob").ap())
            for t_, d_ in ((xq_0, xqT), (xk_0, xkT), (xv_0, xvT)):
                nc.sync.dma_start(
                    out=t_[:, :, GP:LGP],
                    in_=d_[:, GP:LGP].rearrange("(h p) a -> p h a", p=128))

            wct_sb = blob_sb[0:GP, C_WCT:C_WCT + 125]
            wit_sb = blob_sb[0:GP, C_WIT:C_WIT + 125]
            zrow = cpool.tile([1, 512], BF16, name="zrow")
            nc.vector.memset(zrow, 0.0)

            u_ps = psU.tile([128, 512], F32, name="u_state")

            # persistent mpi stationaries (manual triple-buffer): zero columns
            # are memset once; the per-superchunk mul only rewrites cols 5:130
            mpi_bufs = []
            for i_ in range(4):
                mb_ = spool.tile([125, 250], BF16, name=f"mpi_{i_}",
                                 tag=f"mpi_{i_}", bufs=1)
                nc.vector.memset(mb_[:, 0:5], 0.0)
                nc.vector.memset(mb_[:, 130:250], 0.0)
                mpi_bufs.append(mb_)

            xg = {"x": (xq_0, xk_0, xv_0)}

            def load_group(gidx):
                gsl = slice(gidx * LGP, (gidx + 1) * LGP)
                xq_g = xpool.tile([128, 2, LGP], BF16, name=f"xq_{gidx}", tag="xq")
                xk_g = xpool.tile([128, 2, LGP], BF16, name=f"xk_{gidx}", tag="xk")
                xv_g = xpool.tile([128, 2, LGP], BF16, name=f"xv_{gidx}", tag="xv")
                nc.sync.dma_start(out=xq_g, in_=xqT[:, gsl].rearrange("(h p) a -> p h a", p=128))
                nc.sync.dma_start(out=xk_g, in_=xkT[:, gsl].rearrange("(h p) a -> p h a", p=128))
                nc.sync.dma_start(out=xv_g, in_=xvT[:, gsl].rearrange("(h p) a -> p h a", p=128))
                xg["x"] = (xq_g, xk_g, xv_g)

            def proj_sc(s):
                """A+B: all six projections for superchunk s -> sbuf bf16."""
                gidx, ls = divmod(s, LG)
                if ls == 0 and gidx > 0:
                    load_group(gidx)
                xq_g, xk_g, xv_g = xg["x"]
                lsl = slice(ls * GP, (ls + 1) * GP)

                # A: Q/K dim-major, 4 closed psum groups in one bank:
                # qlo 0:125, qhi 125:250, klo 250:375, khi 375:500
                qk = psQK.tile([128, 500], F32, name=f"qk_{s}", tag="qk")
                for gi, (wlo, x_) in enumerate(
                        ((0, xq_g), (128, xq_g), (256, xk_g), (384, xk_g))):
                    for h in (0, 1):
                        mm(qk[:, gi * 125:(gi + 1) * 125],
                           w_sb[:, h, wlo:wlo + 128], x_[:, h, lsl],
                           start=(h == 0), stop=(h == 1), skip_group_check=True)
                qk_sb = spool.tile([128, 500], BF16, name=f"qk_sb_{s}",
                                   tag="qksb", bufs=3)
                nc.scalar.activation(qk_sb, qk, COPY)

                # B: K/V pos-major: K cols 0:256, V cols 256:512.  The last
                # superchunk's K is only consumed by the (skipped) final
                # state update, so project V alone there.
                kv = psKV.tile([125, 512], F32, name=f"kv_{s}", tag="kv")
                if s < NSC - 1:
                    for h in (0, 1):
                        mm(kv[:, 0:256], xk_g[:, h, lsl], w_sb[:, h, 256:512],
                           start=(h == 0), stop=(h == 1), skip_group_check=True)
                for h in (0, 1):
                    mm(kv[:, 256:512], xv_g[:, h, lsl], w_sb[:, h, 512:768],
                       start=(h == 0), stop=(h == 1), skip_group_check=True)
                kv_sb = spool.tile([125, 512], BF16, name=f"kv_sb_{s}",
                                   tag="kvsb", bufs=4)
                if s < NSC - 1:
                    nc.scalar.activation(kv_sb, kv, COPY)
                else:
                    nc.scalar.activation(kv_sb[:, 256:512], kv[:, 256:512], COPY)
                return dict(qlo=qk_sb[:, 0:125], qhi=qk_sb[:, 125:250],
                            klo=qk_sb[:, 250:375], khi=qk_sb[:, 375:500],
                            k_sb=kv_sb[:, 0:256], v_sb=kv_sb[:, 256:512])

            def pt_sc(s, st, dve_comb=False):
                """F: P~^T = K~ @ Q~^T, then DVE masks + combine."""
                pt_ps = psPT.tile([125, 125], F32, name=f"pt_{s}", tag="pt")
                mm(pt_ps, st["klo"], st["qlo"], start=True, stop=False)
                mm(pt_ps, st["khi"], st["qhi"], start=False, stop=True)

                mpc_sb = spool.tile([125, 125], BF16, name=f"mpc_{s}",
                                    tag="mpc", bufs=2)
                comb_sb = spool.tile([125, 125], BF16, name=f"comb_{s}",
                                     tag="comb", bufs=3)
                mpi_sb = mpi_bufs[s % 4]
                nc.vector.tensor_mul(mpc_sb, pt_ps, wct_sb)
                nc.vector.tensor_mul(mpi_sb[:, 5:130], pt_ps, wit_sb)
                if dve_comb:
                    nc.vector.tensor_add(comb_sb, mpc_sb, mpi_sb[:, 0:125])
                else:
                    nc.gpsimd.tensor_add(comb_sb, mpc_sb, mpi_sb[:, 0:125])
                st["comb"] = comb_sb
                st["mpi"] = mpi_sb

            # --- phase 0: projections for s=0, then U init (zero matmul
            # sets data + has_written bits so state matmuls accumulate) ---
            S = {0: proj_sc(0)}
            UT = {}
            mm(u_ps, zrow[:, 0:128], zrow[:, 0:512],
               start=True, stop=True, skip_group_check=True)

            # Deep pipeline: phase s runs PT(s-1), proj(s), state(s-2),
            # window(s-2).  Every cross-engine product (masks, combine, U
            # copy, kv copy) is consumed a FULL phase after it is produced,
            # so engine jitter never stalls the PE.
            for s in range(1, NSC + 2):
                # F: P~^T + masks for superchunk s-1.  The last PT (NSC-1)
                # is hoisted to the end of phase NSC-1 so the tail window
                # never waits on fresh masks.
                if s <= NSC - 1:
                    pt_sc(s - 1, S[s - 1])

                # A+B for superchunk s
                if s < NSC:
                    S[s] = proj_sc(s)

                sm = s - 2
                # E: state update with K/V of sm (last one needed: NSC-2)
                if 0 <= sm <= NSC - 2:
                    mm(u_ps[:, 0:256], S[sm]["k_sb"][:, 0:128], S[sm]["v_sb"],
                       start=False, stop=True, skip_group_check=True)
                    mm(u_ps[:, 256:512], S[sm]["k_sb"][:, 128:256], S[sm]["v_sb"],
                       start=False, stop=True, skip_group_check=True)
                    UT[sm] = spool.tile([128, 512], BF16, name=f"ut_{sm}", tag="ut", bufs=3)
                    if sm % 2 == 0 and sm < NSC - 3:
                        nc.vector.tensor_copy(UT[sm], u_ps)
                    else:
                        nc.scalar.activation(UT[sm], u_ps, COPY)

                # final-tail matmul early so its copy+DMA overlap window(31)
                if s == NSC + 1:
                    wtf = psW.tile([125, 256], F32, name="wt_final", tag="wt")
                    mm(wtf, S[NSC - 1]["mpi"][:, 125:250], S[NSC - 1]["v_sb"],
                       start=True, stop=True)
                    wallf_sb = spool.tile([5, 256], F32, name="wallf", tag="wallf")
                    nc.vector.tensor_copy(wallf_sb, wtf[0:5])
                    nc.sync.dma_start(out=out[SEQ - B:SEQ], in_=wallf_sb)

                if sm < 0:
                    continue
                # D: window for superchunk sm
                wt = psW.tile([125, 256], F32, name=f"wt_{sm}", tag="wt")
                if sm == 0:
                    mm(wt, S[0]["comb"], S[0]["v_sb"], start=True, stop=True)
                else:
                    mm(wt, S[sm]["comb"], S[sm]["v_sb"], start=True, stop=False)
                    mm(wt, S[sm - 1]["mpi"][:, 125:250], S[sm - 1]["v_sb"],
                       start=False, stop=False)
                    mm(wt, S[sm]["qlo"], UT[sm - 1][:, 0:256],
                       start=False, stop=False)
                    mm(wt, S[sm]["qhi"], UT[sm - 1][:, 256:512],
                       start=False, stop=True)

                # window output (copy engine alternates to balance DVE/Act)
                wall_sb = spool.tile([125, 256], F32, name=f"wall_{sm}",
                                     tag="wall", bufs=3)
                if sm >= NSC - 3:
                    nc.scalar.activation(wall_sb, wt, COPY)
                else:
                    nc.vector.tensor_copy(wall_sb, wt)
                if sm == 0:
                    nc.sync.dma_start(out=out[0:GP - B], in_=wall_sb[B:GP])
                else:
                    nc.sync.dma_start(out=out[sm * GP - B: sm * GP - B + GP],
                                      in_=wall_sb)
                if s == NSC - 1:
                    pt_sc(NSC - 1, S[NSC - 1], dve_comb=True)
                if sm - 2 >= 0:
                    del S[sm - 2]

    return nc


def _col_scales():
    j = np.arange(SEQ) // B          # global chunk index
    sq = (np.float64(g6) ** j).astype(np.float32)
    sk = (np.float64(g6) ** (-j)).astype(np.float32)
    return sq, sk


def prep_core_inputs(xq2d, xk2d, xv2d, wqkv):
    sq, sk = _col_scales()
    bf = ml_dtypes.bfloat16
    return {
        "xqT": np.ascontiguousarray((xq2d.T * sq[None, :]).astype(bf)),
        "xkT": np.ascontiguousarray((xk2d.T * sk[None, :]).astype(bf)),
        "xvT": np.ascontiguousarray(xv2d.T.astype(bf)),
        "wqkv": wqkv,
    }


def make_in_maps(inputs):
    """inputs: dict from setup_inputs (full batch). Returns per-core in_maps."""
    xq, xk, xv = inputs["xq"], inputs["xk"], inputs["xv"]
    wqkv = np.ascontiguousarray(np.concatenate(
        [np.asarray(inputs["Wq"], dtype=np.float32),
         np.asarray(inputs["Wk"], dtype=np.float32),
         np.asarray(inputs["Wv"], dtype=np.float32)],
        axis=1).astype(ml_dtypes.bfloat16))
    in_maps = []
    for b in range(8):
        in_maps.append(prep_core_inputs(
            np.asarray(xq[b], dtype=np.float32),
            np.asarray(xk[b], dtype=np.float32),
            np.asarray(xv[b], dtype=np.float32), wqkv))
    return in_maps


_NC_CACHE = {}


def _get_nc():
    if "nc" not in _NC_CACHE:
        from concourse import bacc
        nc = bacc.Bacc("TRN2", target_bir_lowering=False, debug=False)
        build_kernel(nc)
        nc.compile()
        _NC_CACHE["nc"] = nc
    return _NC_CACHE["nc"]


def run(inputs, trace=False, **kwargs):
    """Run on 8 NeuronCores; returns (output [8,4000,256], BassKernelResults)."""
    from concourse.bass_utils import run_bass_kernel_spmd

    nc = _get_nc()
    in_maps = make_in_maps(inputs)
    res = run_bass_kernel_spmd(nc, in_maps, core_ids=list(range(8)),
                               trace=trace, **kwargs)
    out = np.stack([r["out"] for r in res.results], axis=0)
    return out, res


def kernel(**inputs) -> np.ndarray:
    out, _ = run(inputs)
    return out
